# revision 31
# baseline (speedup 1.0000x reference)
"""Trainium2 Bass kernel for batched two-layer-MLP attention.

Reference semantics (per batch b):
    x  = sequence[:, b, :]                        # [S, D]
    K  = tanh(tanh(x @ Kw1.T) @ Kw2.T)
    Q  = tanh(tanh(x @ Qw1.T) @ Qw2.T)
    W  = softmax(K @ Q.T / sqrt(D), axis=-1)      # [S, S]
    out[:, b, :] = W @ x
Sharding: data-parallel over batch (B=8 -> 8 NeuronCores), weights replicated.

Precision plan (rel-err budget 2e-2, measured ~1.8e-2):
  - scores K@Q.T entirely in fp8e4 DoubleRow (2x PE rate): logit quantization
    error is damped by the 1/32 softmax scale.
  - MLP layers split-K: k-tiles 0,1 of every 1024-deep contraction run as one
    fp8 DoubleRow matmul (inputs + weights fp8, weights pre-scaled x32 on the
    host, 1/32 folded into the tanh activation scale); k-tiles 2-7 stay bf16.
  - attended W@x stays bf16 (fp8 there costs ~2.6% error - too much).

Layout strategy per core:
  - transposed everything: weights pre-arranged [p, j, k, c] host-side so each
    j-block loads with one partition-contiguous DMA; x.T bf16 k-tiles 2-7 in
    xt6/xh, x.T fp8 k-tiles 0,1 in xt8 (tiny, lands early -> early PE start)
  - MLP outputs stay transposed; hidden j-tiles 0,1 are written fp8 (h8) for
    the next layer's DoubleRow, j-tiles 2-7 bf16 (h6)
  - scores SC[s, t] = sum_d Kt[d,s] * Qt[d,t] via fp8 DR over d-tile pairs
  - softmax along free axis; exp's accum_out gives row sums for free
  - each exp(SC) row-block is transposed with ONE xbar DMA-transpose (bf16)
    into lhsT layout for attended = Wt.T @ x with rhs = xn [S, D]
  - 1/rowsum folded into the PSUM->SBUF copy of the output

Scheduling: HAM warmup matmuls fill the initial DMA wait; phase B's scores
PSUM pool is opened early so its banks don't overlap phase A's; phase B is
software-pipelined one row-block ahead.
"""

import numpy as np
import ml_dtypes

import concourse.bacc as bacc
import concourse.tile as tile
from concourse import mybir
from concourse.bass_utils import run_bass_kernel_spmd

P = 128          # partitions
S = 2048         # sequence length
D = 1024         # model dim
B = 8            # batch (one per core)
ST = S // P      # 16 s-tiles
DT = D // P      # 8 d-tiles
NF = 512         # psum free width (one bank of fp32)
SN = S // NF     # 4 score free-chunks
DN = D // NF     # 2 output free-chunks
KF8 = 2          # leading k-tiles per contraction done in fp8 DoubleRow
KB = DT - KF8    # bf16 k-tiles (6)
BF = mybir.dt.bfloat16
F8 = mybir.dt.float8e4
F32 = mybir.dt.float32
SCALE = 1.0 / np.sqrt(np.float32(D))
W8SCALE = 32.0   # host pre-scale on fp8 weight tiles; undone in tanh scale

# which MLP layers use the fp8 split-K head (1-4); tune against error budget
FP8_LAYERS = (1, 3, 4)

AX = mybir.AxisListType.X
AF = mybir.ActivationFunctionType
DR = mybir.MatmulPerfMode.DoubleRow


def build_nc():
    nc = bacc.Bacc("TRN2", target_bir_lowering=False)

    # x.T bf16 k-tiles 2-7: [p, kk, s]; n=0 chunk k-halved in xh for
    # mid-group dep granularity
    xt6_d = nc.dram_tensor("xt6", [P, KB, S], BF, kind="ExternalInput")
    xh_d = nc.dram_tensor("xh", [P, 2, KB // 2, NF], BF, kind="ExternalInput")
    # x.T k-tiles 0,1: fp8 (tiny, lands early) + bf16 fallback for unsplit
    xt8_d = nc.dram_tensor("xt8", [P, KF8, S], F8, kind="ExternalInput")
    xb2_d = nc.dram_tensor("xb2", [P, KF8, S], BF, kind="ExternalInput")
    xn_d = nc.dram_tensor("xn", [S, D], BF, kind="ExternalInput")
    # weights [p, j, k, c], all pre-scaled x W8SCALE host-side:
    # bf16 part k=2..7, fp8 part k=0,1, bf16 fallback part k=0,1
    WSHB = [P, DT, KB, P]
    WSH8 = [P, DT, KF8, P]
    w_d, w8_d, wb2_d = {}, {}, {}
    for nm in ("wk1", "wk2", "wq1", "wq2"):
        w_d[nm] = nc.dram_tensor(nm, WSHB, BF, kind="ExternalInput")
        w8_d[nm] = nc.dram_tensor(nm + "_8", WSH8, F8, kind="ExternalInput")
        wb2_d[nm] = nc.dram_tensor(nm + "_2", WSH8, BF, kind="ExternalInput")
    out_d = nc.dram_tensor("out", [S, D], F32, kind="ExternalOutput")

    from contextlib import ExitStack

    with tile.TileContext(nc) as tc, ExitStack() as ctx:
        # ---- persistent SBUF arrays (live across both phases) ----
        pers = ctx.enter_context(tc.tile_pool(name="pers", bufs=1))
        xn_sb = pers.tile([P, ST, D], BF)     # x normal: [t-part, t-tile, d]
        # K.T / Q.T in fp8 for the DoubleRow scores matmul
        kt_sb = pers.tile([P, DT, S], F8)     # K.T: [d-part, d-tile, s]
        qt_n = [pers.tile([P, DT, NF], F8, tag=f"qt{n}", name=f"qt{n}")
                for n in range(SN)]

        # scores PSUM pool opened before phase A so its banks are disjoint
        # from the MLP pool (no released-pool dependency)
        psc = ctx.enter_context(tc.tile_pool(name="psum_sc", bufs=3, space="PSUM"))

        # ---- phase A: the four MLP layers ----
        with tc.tile_pool(name="phase_a", bufs=1) as pa, \
             tc.tile_pool(name="wpool", bufs=2) as wp, \
             tc.tile_pool(name="psum_mlp", bufs=4, space="PSUM") as pm:
            KH = KB // 2
            # layer-1/3 bf16 inputs: n0 in two k-halves, n1-3 whole
            xh_f = [pa.tile([P, KH, NF], BF, tag=f"xh{h}", name=f"xh{h}")
                    for h in range(2)]
            xt6_n = [pa.tile([P, KB, NF], BF, tag=f"x6{n}", name=f"x6{n}")
                     for n in range(1, SN)]
            # layer-1/3 fp8 inputs, per n-chunk
            xt8_n = [pa.tile([P, KF8, NF], F8, tag=f"x8{n}", name=f"x8{n}")
                     for n in range(SN)]

            h6_sb = pa.tile([P, KB, S], BF)    # hidden j-tiles 2-7 (reused K/Q)
            h8_sb = pa.tile([P, KF8, S], F8)   # hidden j-tiles 0,1 (split)
            # bf16 fallbacks for k-tiles 0,1 when a layer is not split
            need_xb2 = not ({1, 3} <= set(FP8_LAYERS))
            need_hb2 = not ({2, 4} <= set(FP8_LAYERS))
            xb2_sb = (pa.tile([P, KF8, S], BF, name="xb2_sb")
                      if need_xb2 else None)
            hb2_sb = (pa.tile([P, KF8, S], BF, name="hb2_sb")
                      if need_hb2 else None)

            # HAM warmup: throwaway matmuls while the first input DMAs fly
            warm_sb = pa.tile([P, NF], BF)
            nc.vector.memset(warm_sb, 0.0)
            warm_ps = pm.tile([P, NF], F32, tag="warm", bufs=1)
            NWARM = 0
            for i in range(NWARM):
                nc.tensor.matmul(warm_ps, warm_sb[:, 0:P], warm_sb,
                                 start=(i == 0), stop=(i == NWARM - 1))

            def mlp_layer(lidx, src6, src8, srcb2, wname, dst,
                          xdma=None, first=False):
                """dst[j, s] = tanh(sum_k w[k, j].T @ src[k, s]); transposed
                layout. k-tiles 0,1 go as one fp8 DoubleRow matmul when lidx
                is in FP8_LAYERS, else two bf16 matmuls; k-tiles 2-7 always
                bf16. Weights pre-scaled x W8SCALE, undone in the tanh."""
                use8 = lidx in FP8_LAYERS
                wb_j = [wp.tile([P, KB, P], BF, tag=f"w{j}", name=f"w{j}")
                        for j in range(DT)]
                if use8:
                    wh_j = [wp.tile([P, KF8, P], F8, tag=f"w8{j}",
                                    name=f"w8{j}") for j in range(DT)]
                    wh_dram = w8_d[wname]
                else:
                    wh_j = [wp.tile([P, KF8, P], BF, tag=f"wb2{j}",
                                    name=f"wb2{j}") for j in range(DT)]
                    wh_dram = wb2_d[wname]
                # interleave per j so wb_j[0] lands before w8_j[7] etc.
                for j in range(DT):
                    nc.scalar.dma_start(out=wh_j[j], in_=wh_dram[:, j, :, :])
                    nc.scalar.dma_start(out=wb_j[j], in_=w_d[wname][:, j, :, :])
                if lidx == 1:
                    # layer 1: pull in the x tiles (layer 3 reuses them)
                    xt8_r = xt8_d.rearrange("p k (n f) -> p k n f", f=NF)
                    xt6_r = xt6_d.rearrange("p k (n f) -> p k n f", f=NF)
                    if 1 in FP8_LAYERS or 3 in FP8_LAYERS:
                        nc.sync.dma_start(out=xt8_n[0], in_=xt8_r[:, :, 0, :])
                    if need_xb2:
                        xb2_r = xb2_d.rearrange("p k (n f) -> p k n f", f=NF)
                        nc.sync.dma_start(out=xb2_sb[:, :, 0:NF],
                                          in_=xb2_r[:, :, 0, :])
                    for h in range(2):
                        nc.sync.dma_start(out=xh_f[h], in_=xh_d[:, h, :, :])
                    for n in range(1, SN):
                        if 1 in FP8_LAYERS or 3 in FP8_LAYERS:
                            nc.sync.dma_start(out=xt8_n[n],
                                              in_=xt8_r[:, :, n, :])
                        if need_xb2:
                            nc.sync.dma_start(
                                out=xb2_sb[:, :, n * NF:(n + 1) * NF],
                                in_=xb2_r[:, :, n, :])
                        # two k-halves for finer dep granularity
                        nc.sync.dma_start(out=xt6_n[n - 1][:, 0:KH, :],
                                          in_=xt6_r[:, 0:KH, n, :])
                        nc.sync.dma_start(out=xt6_n[n - 1][:, KH:KB, :],
                                          in_=xt6_r[:, KH:KB, n, :])
                if xdma is not None:
                    xdma()

                def bf_slice(n, kk):      # kk in 0..KB-1 (k-tile kk+2)
                    if first:
                        if n == 0:
                            return xh_f[kk // KH][:, kk % KH, :]
                        return xt6_n[n - 1][:, kk, :]
                    return src6[:, kk, n * NF:(n + 1) * NF]

                def f8_slice(n):
                    if first:
                        return xt8_n[n]
                    return src8[:, :, n * NF:(n + 1) * NF]

                def b2_slice(n):
                    src = xb2_sb if first else srcb2
                    return src[:, :, n * NF:(n + 1) * NF]

                for n, j in ([(n, j) for n in range(SN) for j in range(DT)]
                             if first else
                             [(n, j) for j in range(DT) for n in range(SN)]):
                    ps = pm.tile([P, NF], F32, tag="mlp")
                    if use8:
                        nc.tensor.matmul(ps, wh_j[j], f8_slice(n),
                                         start=True, stop=False, perf_mode=DR)
                    else:
                        for kk in range(KF8):
                            nc.tensor.matmul(ps, wh_j[j][:, kk, :],
                                             b2_slice(n)[:, kk, :],
                                             start=(kk == 0), stop=False)
                    for kk in range(KB):
                        nc.tensor.matmul(ps, wb_j[j][:, kk, :], bf_slice(n, kk),
                                         start=False, stop=(kk == KB - 1))
                    if isinstance(dst, list):
                        dslice = dst[n][:, j, :]
                    elif len(dst) == 1:
                        dslice = dst[0][:, j, n * NF:(n + 1) * NF]
                    elif j < KF8:
                        dslice = dst[0][:, j, n * NF:(n + 1) * NF]
                    else:
                        dslice = dst[1][:, j - KF8, n * NF:(n + 1) * NF]
                    nc.scalar.activation(out=dslice, in_=ps, func=AF.Tanh,
                                         scale=1.0 / W8SCALE)

            def load_xn():
                xn_r = xn_d.rearrange("(t p) d -> p t d", p=P)
                for t in range(0, ST, 4):
                    nc.sync.dma_start(out=xn_sb[:, t:t + 4, :],
                                      in_=xn_r[:, t:t + 4, :])

            h_for_2 = h8_sb if 2 in FP8_LAYERS else hb2_sb
            h_for_4 = h8_sb if 4 in FP8_LAYERS else hb2_sb
            mlp_layer(1, None, None, None, "wk1", (h_for_2, h6_sb),
                      first=True)
            mlp_layer(2, h6_sb, h8_sb, hb2_sb, "wk2", (kt_sb,))
            mlp_layer(3, None, None, None, "wq1", (h_for_4, h6_sb),
                      xdma=load_xn, first=True)
            mlp_layer(4, h6_sb, h8_sb, hb2_sb, "wq2", qt_n)

        # ---- phase B: scores -> softmax -> transpose -> attended ----
        with tc.tile_pool(name="wexp", bufs=3) as wexp_pool, \
             tc.tile_pool(name="wtT", bufs=3) as wtT_pool, \
             tc.tile_pool(name="sums", bufs=6) as sums_pool, \
             tc.tile_pool(name="outst", bufs=2) as out_pool, \
             tc.tile_pool(name="psum_at", bufs=3, space="PSUM") as pat:

            def scores_softmax_transpose(i):
                wexp = wexp_pool.tile([P, S], BF, tag="wexp")
                sums = sums_pool.tile([P, SN], F32, tag="sums")
                for n in range(SN):
                    ps = psc.tile([P, NF], F32, tag="sc")
                    for k in range(0, DT, 2):
                        nc.tensor.matmul(
                            ps,
                            kt_sb[:, k:k + 2, i * P:(i + 1) * P],
                            qt_n[n][:, k:k + 2, :],
                            start=(k == 0),
                            stop=(k == DT - 2),
                            perf_mode=DR,
                        )
                    # scores are bounded (|sc/32| < ~3): exp without max-shift
                    nc.scalar.activation(
                        out=wexp[:, n * NF:(n + 1) * NF],
                        in_=ps,
                        func=AF.Exp,
                        scale=float(SCALE),
                        accum_out=sums[:, n:n + 1],
                    )
                rcp = sums_pool.tile([P, 1], F32, tag="rcp")
                nc.vector.reduce_sum(rcp, sums, axis=AX)
                nc.vector.reciprocal(rcp, rcp)
                wtT = wtT_pool.tile([P, ST, P], BF, tag="wtT")
                nc.scalar.dma_start_transpose(out=wtT, in_=wexp)
                return wtT, rcp

            def attended(i, wtT, rcp, last=False):
                outst = out_pool.tile([P, D], F32, tag="outst")
                for n in range(DN):
                    ps = pat.tile([P, NF], F32, tag="at")
                    for t in range(ST):
                        nc.tensor.matmul(
                            ps,
                            wtT[:, t, :],
                            xn_sb[:, t, n * NF:(n + 1) * NF],
                            start=(t == 0),
                            stop=(t == ST - 1),
                        )
                    nc.scalar.mul(outst[:, n * NF:(n + 1) * NF], ps, rcp)
                    nc.sync.dma_start(
                        out=out_d[i * P:(i + 1) * P, n * NF:(n + 1) * NF],
                        in_=outst[:, n * NF:(n + 1) * NF],
                    )

            # depth-2 software pipeline: attended(i-2) is emitted after
            # scores(i), giving each transpose a full block of slack
            pipe = []
            for i in range(ST):
                pipe.append(scores_softmax_transpose(i))
                if i >= 2:
                    attended(i - 2, *pipe[i - 2])
            attended(ST - 2, *pipe[ST - 2])
            attended(ST - 1, *pipe[ST - 1], last=True)

    nc.compile()
    return nc


_NC = None


def _get_nc():
    global _NC
    if _NC is None:
        _NC = build_nc()
    return _NC


def _prep_w(w):
    """[d_out, d_in] f32 -> (bf16 [p,j,k2-7,c], fp8 [p,j,k0-1,c],
    bf16 [p,j,k0-1,c]).

    ALL parts are pre-scaled by W8SCALE (so the fp8 tiles sit in e4m3's
    normal range) and the whole psum is uniformly scaled; the tanh
    activation applies 1/W8SCALE. bf16 loses nothing to the scaling."""
    bf16 = ml_dtypes.bfloat16
    f8 = ml_dtypes.float8_e4m3
    wt = np.asarray(w).T.reshape(DT, P, DT, P).transpose(1, 2, 0, 3)  # p j k c
    wt = wt * W8SCALE
    wb = np.ascontiguousarray(wt[:, :, KF8:, :]).astype(bf16)
    w8 = np.ascontiguousarray(wt[:, :, :KF8, :]).astype(f8)
    wb2 = np.ascontiguousarray(wt[:, :, :KF8, :]).astype(bf16)
    return wb, w8, wb2


def make_in_maps(sequence, Kw1, Kw2, Qw1, Qw2):
    bf16 = ml_dtypes.bfloat16
    f8 = ml_dtypes.float8_e4m3
    seq = np.ascontiguousarray(np.transpose(np.asarray(sequence), (1, 0, 2)))
    ws = {}
    for nm, w in (("wk1", Kw1), ("wk2", Kw2), ("wq1", Qw1), ("wq2", Qw2)):
        wb, w8, wb2 = _prep_w(w)
        ws[nm] = wb
        ws[nm + "_8"] = w8
        ws[nm + "_2"] = wb2
    in_maps = []
    for b in range(B):
        xb = seq[b]                                   # [S, D] f32
        xt = np.ascontiguousarray(xb.T)               # [D, S] f32
        # bf16 k-tiles 2-7: [p, kk, s]
        xt6 = np.ascontiguousarray(
            xt[KF8 * P:].reshape(KB, P, S).transpose(1, 0, 2)).astype(bf16)
        # n=0 chunk k-halved: [p, h, kh, f]
        xh = np.ascontiguousarray(
            xt6[:, :, 0:NF].reshape(P, 2, KB // 2, NF))
        # k-tiles 0,1: [p, k, s], fp8 + bf16 fallback
        xklo = xt[:KF8 * P].reshape(KF8, P, S).transpose(1, 0, 2)
        xt8 = np.ascontiguousarray(xklo).astype(f8)
        xb2 = np.ascontiguousarray(xklo).astype(bf16)
        m = {"xn": xb.astype(bf16), "xt6": xt6, "xh": xh, "xt8": xt8,
             "xb2": xb2}
        m.update(ws)
        in_maps.append(m)
    return in_maps


def kernel(sequence, Kw1, Kw2, Qw1, Qw2):
    nc = _get_nc()
    in_maps = make_in_maps(sequence, Kw1, Kw2, Qw1, Qw2)
    res = run_bass_kernel_spmd(nc, in_maps, core_ids=list(range(B)))
    out = np.stack([res.results[b]["out"] for b in range(B)], axis=1)
    return out.astype(np.float32)


# revision 32
# speedup vs baseline: 1.1893x; 1.1893x over previous
"""Trainium2 Bass kernel for batched two-layer-MLP attention.

Reference semantics (per batch b):
    x  = sequence[:, b, :]                        # [S, D]
    K  = tanh(tanh(x @ Kw1.T) @ Kw2.T)
    Q  = tanh(tanh(x @ Qw1.T) @ Qw2.T)
    W  = softmax(K @ Q.T / sqrt(D), axis=-1)      # [S, S]
    out[:, b, :] = W @ x
Sharding: data-parallel over batch (B=8 -> 8 NeuronCores), weights replicated.

Precision plan (rel-err budget 2e-2, measured ~1.8e-2):
  - scores K@Q.T entirely in fp8e4 DoubleRow (2x PE rate): logit quantization
    error is damped by the 1/32 softmax scale.
  - MLP layers split-K: k-tiles 0,1 of every 1024-deep contraction run as one
    fp8 DoubleRow matmul (inputs + weights fp8, weights pre-scaled x32 on the
    host, 1/32 folded into the tanh activation scale); k-tiles 2-7 stay bf16.
  - attended W@x stays bf16 (fp8 there costs ~2.6% error - too much).

Layout strategy per core:
  - transposed everything: weights pre-arranged [p, j, k, c] host-side so each
    j-block loads with one partition-contiguous DMA; x.T bf16 k-tiles 2-7 in
    xt6/xh, x.T fp8 k-tiles 0,1 in xt8 (tiny, lands early -> early PE start)
  - MLP outputs stay transposed; hidden j-tiles 0,1 are written fp8 (h8) for
    the next layer's DoubleRow, j-tiles 2-7 bf16 (h6)
  - scores SC[s, t] = sum_d Kt[d,s] * Qt[d,t] via fp8 DR over d-tile pairs
  - softmax along free axis; exp's accum_out gives row sums for free
  - each exp(SC) row-block is transposed with ONE xbar DMA-transpose (bf16)
    into lhsT layout for attended = Wt.T @ x with rhs = xn [S, D]
  - 1/rowsum folded into the PSUM->SBUF copy of the output

Scheduling: HAM warmup matmuls fill the initial DMA wait; phase B's scores
PSUM pool is opened early so its banks don't overlap phase A's; phase B is
software-pipelined one row-block ahead.
"""

import numpy as np
import ml_dtypes

import concourse.bacc as bacc
import concourse.tile as tile
from concourse import mybir
from concourse.bass_utils import run_bass_kernel_spmd

P = 128          # partitions
S = 2048         # sequence length
D = 1024         # model dim
B = 8            # batch (one per core)
ST = S // P      # 16 s-tiles
DT = D // P      # 8 d-tiles
NF = 512         # psum free width (one bank of fp32)
SN = S // NF     # 4 score free-chunks
DN = D // NF     # 2 output free-chunks
KF8 = 2          # leading k-tiles per contraction done in fp8 DoubleRow
KB = DT - KF8    # bf16 k-tiles (6)
BF = mybir.dt.bfloat16
F8 = mybir.dt.float8e4
F32 = mybir.dt.float32
SCALE = 1.0 / np.sqrt(np.float32(D))
W8SCALE = 32.0   # host pre-scale on fp8 weight tiles; undone in tanh scale

# which MLP layers use the fp8 split-K head (1-4); tune against error budget
FP8_LAYERS = (1, 3, 4)

AX = mybir.AxisListType.X
AF = mybir.ActivationFunctionType
DR = mybir.MatmulPerfMode.DoubleRow


def build_nc():
    nc = bacc.Bacc("TRN2", target_bir_lowering=False)

    # x.T bf16 k-tiles 2-7: [p, kk, s]; n=0 chunk k-halved in xh for
    # mid-group dep granularity
    xt6_d = nc.dram_tensor("xt6", [P, KB, S], BF, kind="ExternalInput")
    xh_d = nc.dram_tensor("xh", [P, 2, KB // 2, NF], BF, kind="ExternalInput")
    # x.T k-tiles 0,1: fp8 (tiny, lands early) + bf16 fallback for unsplit
    xt8_d = nc.dram_tensor("xt8", [P, KF8, S], F8, kind="ExternalInput")
    xb2_d = nc.dram_tensor("xb2", [P, KF8, S], BF, kind="ExternalInput")
    xn_d = nc.dram_tensor("xn", [S, D], BF, kind="ExternalInput")
    # weights [p, j, k, c], all pre-scaled x W8SCALE host-side:
    # bf16 part k=2..7, fp8 part k=0,1, bf16 fallback part k=0,1
    WSHB = [P, DT, KB, P]
    WSH8 = [P, DT, KF8, P]
    w_d, w8_d, wb2_d = {}, {}, {}
    for nm in ("wk1", "wk2", "wq1", "wq2"):
        w_d[nm] = nc.dram_tensor(nm, WSHB, BF, kind="ExternalInput")
        w8_d[nm] = nc.dram_tensor(nm + "_8", WSH8, F8, kind="ExternalInput")
        wb2_d[nm] = nc.dram_tensor(nm + "_2", WSH8, BF, kind="ExternalInput")
    out_d = nc.dram_tensor("out", [S, D], F32, kind="ExternalOutput")

    from contextlib import ExitStack

    with tile.TileContext(nc) as tc, ExitStack() as ctx:
        # ---- persistent SBUF arrays (live across both phases) ----
        pers = ctx.enter_context(tc.tile_pool(name="pers", bufs=1))
        xn_sb = pers.tile([P, ST, D], BF)     # x normal: [t-part, t-tile, d]
        # K.T / Q.T in fp8 for the DoubleRow scores matmul
        kt_sb = pers.tile([P, DT, S], F8)     # K.T: [d-part, d-tile, s]
        qt_n = [pers.tile([P, DT, NF], F8, tag=f"qt{n}", name=f"qt{n}")
                for n in range(SN)]

        # scores PSUM pool opened before phase A so its banks are disjoint
        # from the MLP pool (no released-pool dependency)
        psc = ctx.enter_context(tc.tile_pool(name="psum_sc", bufs=3, space="PSUM"))

        # ---- phase A: the four MLP layers ----
        with tc.tile_pool(name="phase_a", bufs=1) as pa, \
             tc.tile_pool(name="wpool", bufs=2) as wp, \
             tc.tile_pool(name="psum_mlp", bufs=4, space="PSUM") as pm:
            KH = KB // 2
            # layer-1/3 bf16 inputs: n0 in two k-halves, n1-3 whole
            xh_f = [pa.tile([P, KH, NF], BF, tag=f"xh{h}", name=f"xh{h}")
                    for h in range(2)]
            xt6_n = [pa.tile([P, KB, NF], BF, tag=f"x6{n}", name=f"x6{n}")
                     for n in range(1, SN)]
            # layer-1/3 fp8 inputs, per n-chunk
            xt8_n = [pa.tile([P, KF8, NF], F8, tag=f"x8{n}", name=f"x8{n}")
                     for n in range(SN)]

            h6_sb = pa.tile([P, KB, S], BF)    # hidden j-tiles 2-7 (reused K/Q)
            h8_sb = pa.tile([P, KF8, S], F8)   # hidden j-tiles 0,1 (split)
            # bf16 fallbacks for k-tiles 0,1 when a layer is not split
            need_xb2 = not ({1, 3} <= set(FP8_LAYERS))
            need_hb2 = not ({2, 4} <= set(FP8_LAYERS))
            xb2_sb = (pa.tile([P, KF8, S], BF, name="xb2_sb")
                      if need_xb2 else None)
            hb2_sb = (pa.tile([P, KF8, S], BF, name="hb2_sb")
                      if need_hb2 else None)

            # HAM warmup: throwaway matmuls while the first input DMAs fly
            warm_sb = pa.tile([P, NF], BF)
            nc.vector.memset(warm_sb, 0.0)
            warm_ps = pm.tile([P, NF], F32, tag="warm", bufs=1)
            NWARM = 5
            for i in range(NWARM):
                nc.tensor.matmul(warm_ps, warm_sb[:, 0:P], warm_sb,
                                 start=(i == 0), stop=(i == NWARM - 1))

            def mlp_layer(lidx, src6, src8, srcb2, wname, dst,
                          xdma=None, first=False):
                """dst[j, s] = tanh(sum_k w[k, j].T @ src[k, s]); transposed
                layout. k-tiles 0,1 go as one fp8 DoubleRow matmul when lidx
                is in FP8_LAYERS, else two bf16 matmuls; k-tiles 2-7 always
                bf16. Weights pre-scaled x W8SCALE, undone in the tanh."""
                use8 = lidx in FP8_LAYERS
                wb_j = [wp.tile([P, KB, P], BF, tag=f"w{j}", name=f"w{j}")
                        for j in range(DT)]
                if use8:
                    wh_j = [wp.tile([P, KF8, P], F8, tag=f"w8{j}",
                                    name=f"w8{j}") for j in range(DT)]
                    wh_dram = w8_d[wname]
                else:
                    wh_j = [wp.tile([P, KF8, P], BF, tag=f"wb2{j}",
                                    name=f"wb2{j}") for j in range(DT)]
                    wh_dram = wb2_d[wname]
                # interleave per j so wb_j[0] lands before w8_j[7] etc.
                for j in range(DT):
                    nc.scalar.dma_start(out=wh_j[j], in_=wh_dram[:, j, :, :])
                    nc.scalar.dma_start(out=wb_j[j], in_=w_d[wname][:, j, :, :])
                if lidx == 1:
                    # layer 1: pull in the x tiles (layer 3 reuses them)
                    xt8_r = xt8_d.rearrange("p k (n f) -> p k n f", f=NF)
                    xt6_r = xt6_d.rearrange("p k (n f) -> p k n f", f=NF)
                    if 1 in FP8_LAYERS or 3 in FP8_LAYERS:
                        nc.sync.dma_start(out=xt8_n[0], in_=xt8_r[:, :, 0, :])
                    if need_xb2:
                        xb2_r = xb2_d.rearrange("p k (n f) -> p k n f", f=NF)
                        nc.sync.dma_start(out=xb2_sb[:, :, 0:NF],
                                          in_=xb2_r[:, :, 0, :])
                    for h in range(2):
                        nc.sync.dma_start(out=xh_f[h], in_=xh_d[:, h, :, :])
                    for n in range(1, SN):
                        if 1 in FP8_LAYERS or 3 in FP8_LAYERS:
                            nc.sync.dma_start(out=xt8_n[n],
                                              in_=xt8_r[:, :, n, :])
                        if need_xb2:
                            nc.sync.dma_start(
                                out=xb2_sb[:, :, n * NF:(n + 1) * NF],
                                in_=xb2_r[:, :, n, :])
                        # two k-halves for finer dep granularity
                        nc.sync.dma_start(out=xt6_n[n - 1][:, 0:KH, :],
                                          in_=xt6_r[:, 0:KH, n, :])
                        nc.sync.dma_start(out=xt6_n[n - 1][:, KH:KB, :],
                                          in_=xt6_r[:, KH:KB, n, :])
                if xdma is not None:
                    xdma()

                def bf_slice(n, kk):      # kk in 0..KB-1 (k-tile kk+2)
                    if first:
                        if n == 0:
                            return xh_f[kk // KH][:, kk % KH, :]
                        return xt6_n[n - 1][:, kk, :]
                    return src6[:, kk, n * NF:(n + 1) * NF]

                def f8_slice(n):
                    if first:
                        return xt8_n[n]
                    return src8[:, :, n * NF:(n + 1) * NF]

                def b2_slice(n):
                    src = xb2_sb if first else srcb2
                    return src[:, :, n * NF:(n + 1) * NF]

                for n, j in ([(n, j) for n in range(SN) for j in range(DT)]
                             if first else
                             [(n, j) for j in range(DT) for n in range(SN)]):
                    ps = pm.tile([P, NF], F32, tag="mlp")
                    if use8:
                        nc.tensor.matmul(ps, wh_j[j], f8_slice(n),
                                         start=True, stop=False, perf_mode=DR)
                    else:
                        for kk in range(KF8):
                            nc.tensor.matmul(ps, wh_j[j][:, kk, :],
                                             b2_slice(n)[:, kk, :],
                                             start=(kk == 0), stop=False)
                    for kk in range(KB):
                        nc.tensor.matmul(ps, wb_j[j][:, kk, :], bf_slice(n, kk),
                                         start=False, stop=(kk == KB - 1))
                    if isinstance(dst, list):
                        dslice = dst[n][:, j, :]
                    elif len(dst) == 1:
                        dslice = dst[0][:, j, n * NF:(n + 1) * NF]
                    elif j < KF8:
                        dslice = dst[0][:, j, n * NF:(n + 1) * NF]
                    else:
                        dslice = dst[1][:, j - KF8, n * NF:(n + 1) * NF]
                    nc.scalar.activation(out=dslice, in_=ps, func=AF.Tanh,
                                         scale=1.0 / W8SCALE)

            def load_xn():
                xn_r = xn_d.rearrange("(t p) d -> p t d", p=P)
                for t in range(0, ST, 4):
                    nc.sync.dma_start(out=xn_sb[:, t:t + 4, :],
                                      in_=xn_r[:, t:t + 4, :])

            h_for_2 = h8_sb if 2 in FP8_LAYERS else hb2_sb
            h_for_4 = h8_sb if 4 in FP8_LAYERS else hb2_sb
            mlp_layer(1, None, None, None, "wk1", (h_for_2, h6_sb),
                      first=True)
            mlp_layer(2, h6_sb, h8_sb, hb2_sb, "wk2", (kt_sb,))
            mlp_layer(3, None, None, None, "wq1", (h_for_4, h6_sb),
                      xdma=load_xn, first=True)
            mlp_layer(4, h6_sb, h8_sb, hb2_sb, "wq2", qt_n)

        # ---- phase B: scores -> softmax -> transpose -> attended ----
        with tc.tile_pool(name="wexp", bufs=3) as wexp_pool, \
             tc.tile_pool(name="wtT", bufs=3) as wtT_pool, \
             tc.tile_pool(name="sums", bufs=6) as sums_pool, \
             tc.tile_pool(name="outst", bufs=2) as out_pool, \
             tc.tile_pool(name="psum_at", bufs=3, space="PSUM") as pat:

            def scores_softmax_transpose(i):
                wexp = wexp_pool.tile([P, S], BF, tag="wexp")
                sums = sums_pool.tile([P, SN], F32, tag="sums")
                for n in range(SN):
                    ps = psc.tile([P, NF], F32, tag="sc")
                    for k in range(0, DT, 2):
                        nc.tensor.matmul(
                            ps,
                            kt_sb[:, k:k + 2, i * P:(i + 1) * P],
                            qt_n[n][:, k:k + 2, :],
                            start=(k == 0),
                            stop=(k == DT - 2),
                            perf_mode=DR,
                        )
                    # scores are bounded (|sc/32| < ~3): exp without max-shift
                    nc.scalar.activation(
                        out=wexp[:, n * NF:(n + 1) * NF],
                        in_=ps,
                        func=AF.Exp,
                        scale=float(SCALE),
                        accum_out=sums[:, n:n + 1],
                    )
                rcp = sums_pool.tile([P, 1], F32, tag="rcp")
                nc.vector.reduce_sum(rcp, sums, axis=AX)
                nc.vector.reciprocal(rcp, rcp)
                wtT = wtT_pool.tile([P, ST, P], BF, tag="wtT")
                nc.scalar.dma_start_transpose(out=wtT, in_=wexp)
                return wtT, rcp

            def attended(i, wtT, rcp, last=False):
                outst = out_pool.tile([P, D], F32, tag="outst")
                for n in range(DN):
                    ps = pat.tile([P, NF], F32, tag="at")
                    for t in range(ST):
                        nc.tensor.matmul(
                            ps,
                            wtT[:, t, :],
                            xn_sb[:, t, n * NF:(n + 1) * NF],
                            start=(t == 0),
                            stop=(t == ST - 1),
                        )
                    nc.scalar.mul(outst[:, n * NF:(n + 1) * NF], ps, rcp)
                    nc.sync.dma_start(
                        out=out_d[i * P:(i + 1) * P, n * NF:(n + 1) * NF],
                        in_=outst[:, n * NF:(n + 1) * NF],
                    )

            # depth-2 software pipeline: attended(i-2) is emitted after
            # scores(i), giving each transpose a full block of slack
            pipe = []
            for i in range(ST):
                pipe.append(scores_softmax_transpose(i))
                if i >= 2:
                    attended(i - 2, *pipe[i - 2])
            attended(ST - 2, *pipe[ST - 2])
            attended(ST - 1, *pipe[ST - 1], last=True)

    nc.compile()
    return nc


_NC = None


def _get_nc():
    global _NC
    if _NC is None:
        _NC = build_nc()
    return _NC


def _prep_w(w):
    """[d_out, d_in] f32 -> (bf16 [p,j,k2-7,c], fp8 [p,j,k0-1,c],
    bf16 [p,j,k0-1,c]).

    ALL parts are pre-scaled by W8SCALE (so the fp8 tiles sit in e4m3's
    normal range) and the whole psum is uniformly scaled; the tanh
    activation applies 1/W8SCALE. bf16 loses nothing to the scaling."""
    bf16 = ml_dtypes.bfloat16
    f8 = ml_dtypes.float8_e4m3
    wt = np.asarray(w).T.reshape(DT, P, DT, P).transpose(1, 2, 0, 3)  # p j k c
    wt = wt * W8SCALE
    wb = np.ascontiguousarray(wt[:, :, KF8:, :]).astype(bf16)
    w8 = np.ascontiguousarray(wt[:, :, :KF8, :]).astype(f8)
    wb2 = np.ascontiguousarray(wt[:, :, :KF8, :]).astype(bf16)
    return wb, w8, wb2


def make_in_maps(sequence, Kw1, Kw2, Qw1, Qw2):
    bf16 = ml_dtypes.bfloat16
    f8 = ml_dtypes.float8_e4m3
    seq = np.ascontiguousarray(np.transpose(np.asarray(sequence), (1, 0, 2)))
    ws = {}
    for nm, w in (("wk1", Kw1), ("wk2", Kw2), ("wq1", Qw1), ("wq2", Qw2)):
        wb, w8, wb2 = _prep_w(w)
        ws[nm] = wb
        ws[nm + "_8"] = w8
        ws[nm + "_2"] = wb2
    in_maps = []
    for b in range(B):
        xb = seq[b]                                   # [S, D] f32
        xt = np.ascontiguousarray(xb.T)               # [D, S] f32
        # bf16 k-tiles 2-7: [p, kk, s]
        xt6 = np.ascontiguousarray(
            xt[KF8 * P:].reshape(KB, P, S).transpose(1, 0, 2)).astype(bf16)
        # n=0 chunk k-halved: [p, h, kh, f]
        xh = np.ascontiguousarray(
            xt6[:, :, 0:NF].reshape(P, 2, KB // 2, NF))
        # k-tiles 0,1: [p, k, s], fp8 + bf16 fallback
        xklo = xt[:KF8 * P].reshape(KF8, P, S).transpose(1, 0, 2)
        xt8 = np.ascontiguousarray(xklo).astype(f8)
        xb2 = np.ascontiguousarray(xklo).astype(bf16)
        m = {"xn": xb.astype(bf16), "xt6": xt6, "xh": xh, "xt8": xt8,
             "xb2": xb2}
        m.update(ws)
        in_maps.append(m)
    return in_maps


def kernel(sequence, Kw1, Kw2, Qw1, Qw2):
    nc = _get_nc()
    in_maps = make_in_maps(sequence, Kw1, Kw2, Qw1, Qw2)
    res = run_bass_kernel_spmd(nc, in_maps, core_ids=list(range(B)))
    out = np.stack([res.results[b]["out"] for b in range(B)], axis=1)
    return out.astype(np.float32)


# revision 34
# speedup vs baseline: 1.1927x; 1.0029x over previous
"""Trainium2 Bass kernel for batched two-layer-MLP attention.

Reference semantics (per batch b):
    x  = sequence[:, b, :]                        # [S, D]
    K  = tanh(tanh(x @ Kw1.T) @ Kw2.T)
    Q  = tanh(tanh(x @ Qw1.T) @ Qw2.T)
    W  = softmax(K @ Q.T / sqrt(D), axis=-1)      # [S, S]
    out[:, b, :] = W @ x
Sharding: data-parallel over batch (B=8 -> 8 NeuronCores), weights replicated.

Precision plan (rel-err budget 2e-2, measured ~1.8e-2):
  - scores K@Q.T entirely in fp8e4 DoubleRow (2x PE rate): logit quantization
    error is damped by the 1/32 softmax scale.
  - MLP layers split-K: k-tiles 0,1 of every 1024-deep contraction run as one
    fp8 DoubleRow matmul (inputs + weights fp8, weights pre-scaled x32 on the
    host, 1/32 folded into the tanh activation scale); k-tiles 2-7 stay bf16.
  - attended W@x stays bf16 (fp8 there costs ~2.6% error - too much).

Layout strategy per core:
  - transposed everything: weights pre-arranged [p, j, k, c] host-side so each
    j-block loads with one partition-contiguous DMA; x.T bf16 k-tiles 2-7 in
    xt6/xh, x.T fp8 k-tiles 0,1 in xt8 (tiny, lands early -> early PE start)
  - MLP outputs stay transposed; hidden j-tiles 0,1 are written fp8 (h8) for
    the next layer's DoubleRow, j-tiles 2-7 bf16 (h6)
  - scores SC[s, t] = sum_d Kt[d,s] * Qt[d,t] via fp8 DR over d-tile pairs
  - softmax along free axis; exp's accum_out gives row sums for free
  - each exp(SC) row-block is transposed with ONE xbar DMA-transpose (bf16)
    into lhsT layout for attended = Wt.T @ x with rhs = xn [S, D]
  - 1/rowsum folded into the PSUM->SBUF copy of the output

Scheduling: HAM warmup matmuls fill the initial DMA wait; phase B's scores
PSUM pool is opened early so its banks don't overlap phase A's; phase B is
software-pipelined one row-block ahead.
"""

import numpy as np
import ml_dtypes

import concourse.bacc as bacc
import concourse.tile as tile
from concourse import mybir
from concourse.bass_utils import run_bass_kernel_spmd

P = 128          # partitions
S = 2048         # sequence length
D = 1024         # model dim
B = 8            # batch (one per core)
ST = S // P      # 16 s-tiles
DT = D // P      # 8 d-tiles
NF = 512         # psum free width (one bank of fp32)
SN = S // NF     # 4 score free-chunks
DN = D // NF     # 2 output free-chunks
KF8 = 2          # leading k-tiles per contraction done in fp8 DoubleRow
KB = DT - KF8    # bf16 k-tiles (6)
BF = mybir.dt.bfloat16
F8 = mybir.dt.float8e4
F32 = mybir.dt.float32
SCALE = 1.0 / np.sqrt(np.float32(D))
W8SCALE = 32.0   # host pre-scale on fp8 weight tiles; undone in tanh scale

# which MLP layers use the fp8 split-K head (1-4); tune against error budget
FP8_LAYERS = (1, 3, 4)

AX = mybir.AxisListType.X
AF = mybir.ActivationFunctionType
DR = mybir.MatmulPerfMode.DoubleRow


def build_nc():
    nc = bacc.Bacc("TRN2", target_bir_lowering=False)

    # x.T bf16 k-tiles 2-7: [p, kk, s]; n=0 chunk k-halved in xh for
    # mid-group dep granularity
    xt6_d = nc.dram_tensor("xt6", [P, KB, S], BF, kind="ExternalInput")
    xh_d = nc.dram_tensor("xh", [P, 2, KB // 2, NF], BF, kind="ExternalInput")
    # x.T k-tiles 0,1: fp8 (tiny, lands early) + bf16 fallback for unsplit
    xt8_d = nc.dram_tensor("xt8", [P, KF8, S], F8, kind="ExternalInput")
    xb2_d = nc.dram_tensor("xb2", [P, KF8, S], BF, kind="ExternalInput")
    xn_d = nc.dram_tensor("xn", [S, D], BF, kind="ExternalInput")
    # weights [p, j, k, c], all pre-scaled x W8SCALE host-side:
    # bf16 part k=2..7, fp8 part k=0,1, bf16 fallback part k=0,1
    WSHB = [P, DT, KB, P]
    WSH8 = [P, DT, KF8, P]
    w_d, w8_d, wb2_d = {}, {}, {}
    for nm in ("wk1", "wk2", "wq1", "wq2"):
        w_d[nm] = nc.dram_tensor(nm, WSHB, BF, kind="ExternalInput")
        w8_d[nm] = nc.dram_tensor(nm + "_8", WSH8, F8, kind="ExternalInput")
        wb2_d[nm] = nc.dram_tensor(nm + "_2", WSH8, BF, kind="ExternalInput")
    out_d = nc.dram_tensor("out", [S, D], F32, kind="ExternalOutput")

    from contextlib import ExitStack

    with tile.TileContext(nc) as tc, ExitStack() as ctx:
        # ---- persistent SBUF arrays (live across both phases) ----
        pers = ctx.enter_context(tc.tile_pool(name="pers", bufs=1))
        xn_sb = pers.tile([P, ST, D], BF)     # x normal: [t-part, t-tile, d]
        # K.T / Q.T in fp8 for the DoubleRow scores matmul
        kt_sb = pers.tile([P, DT, S], F8)     # K.T: [d-part, d-tile, s]
        qt_n = [pers.tile([P, DT, NF], F8, tag=f"qt{n}", name=f"qt{n}")
                for n in range(SN)]

        # scores PSUM pool opened before phase A so its banks are disjoint
        # from the MLP pool (no released-pool dependency)
        psc = ctx.enter_context(tc.tile_pool(name="psum_sc", bufs=4, space="PSUM"))

        # ---- phase A: the four MLP layers ----
        with tc.tile_pool(name="phase_a", bufs=1) as pa, \
             tc.tile_pool(name="wpool", bufs=2) as wp, \
             tc.tile_pool(name="psum_mlp", bufs=4, space="PSUM") as pm:
            KH = KB // 2
            # layer-1/3 bf16 inputs: n0 in two k-halves, n1-3 whole
            xh_f = [pa.tile([P, KH, NF], BF, tag=f"xh{h}", name=f"xh{h}")
                    for h in range(2)]
            xt6_n = [pa.tile([P, KB, NF], BF, tag=f"x6{n}", name=f"x6{n}")
                     for n in range(1, SN)]
            # layer-1/3 fp8 inputs, per n-chunk
            xt8_n = [pa.tile([P, KF8, NF], F8, tag=f"x8{n}", name=f"x8{n}")
                     for n in range(SN)]

            h6_sb = pa.tile([P, KB, S], BF)    # hidden j-tiles 2-7 (reused K/Q)
            h8_sb = pa.tile([P, KF8, S], F8)   # hidden j-tiles 0,1 (split)
            # bf16 fallbacks for k-tiles 0,1 when a layer is not split
            need_xb2 = not ({1, 3} <= set(FP8_LAYERS))
            need_hb2 = not ({2, 4} <= set(FP8_LAYERS))
            xb2_sb = (pa.tile([P, KF8, S], BF, name="xb2_sb")
                      if need_xb2 else None)
            hb2_sb = (pa.tile([P, KF8, S], BF, name="hb2_sb")
                      if need_hb2 else None)

            # no warmup: the first real DR matmul's inputs (xt8_n[0] 128KB +
            # w8_j[0] 32KB) land early enough that real work ramps the PE;
            # the freed 8th PSUM bank goes to the scores pool instead

            def mlp_layer(lidx, src6, src8, srcb2, wname, dst,
                          xdma=None, first=False):
                """dst[j, s] = tanh(sum_k w[k, j].T @ src[k, s]); transposed
                layout. k-tiles 0,1 go as one fp8 DoubleRow matmul when lidx
                is in FP8_LAYERS, else two bf16 matmuls; k-tiles 2-7 always
                bf16. Weights pre-scaled x W8SCALE, undone in the tanh."""
                use8 = lidx in FP8_LAYERS
                wb_j = [wp.tile([P, KB, P], BF, tag=f"w{j}", name=f"w{j}")
                        for j in range(DT)]
                if use8:
                    wh_j = [wp.tile([P, KF8, P], F8, tag=f"w8{j}",
                                    name=f"w8{j}") for j in range(DT)]
                    wh_dram = w8_d[wname]
                else:
                    wh_j = [wp.tile([P, KF8, P], BF, tag=f"wb2{j}",
                                    name=f"wb2{j}") for j in range(DT)]
                    wh_dram = wb2_d[wname]
                # interleave per j so wb_j[0] lands before w8_j[7] etc.
                for j in range(DT):
                    nc.scalar.dma_start(out=wh_j[j], in_=wh_dram[:, j, :, :])
                    nc.scalar.dma_start(out=wb_j[j], in_=w_d[wname][:, j, :, :])
                if lidx == 1:
                    # layer 1: pull in the x tiles (layer 3 reuses them)
                    xt8_r = xt8_d.rearrange("p k (n f) -> p k n f", f=NF)
                    xt6_r = xt6_d.rearrange("p k (n f) -> p k n f", f=NF)
                    if 1 in FP8_LAYERS or 3 in FP8_LAYERS:
                        nc.sync.dma_start(out=xt8_n[0], in_=xt8_r[:, :, 0, :])
                    if need_xb2:
                        xb2_r = xb2_d.rearrange("p k (n f) -> p k n f", f=NF)
                        nc.sync.dma_start(out=xb2_sb[:, :, 0:NF],
                                          in_=xb2_r[:, :, 0, :])
                    for h in range(2):
                        nc.sync.dma_start(out=xh_f[h], in_=xh_d[:, h, :, :])
                    for n in range(1, SN):
                        if 1 in FP8_LAYERS or 3 in FP8_LAYERS:
                            nc.sync.dma_start(out=xt8_n[n],
                                              in_=xt8_r[:, :, n, :])
                        if need_xb2:
                            nc.sync.dma_start(
                                out=xb2_sb[:, :, n * NF:(n + 1) * NF],
                                in_=xb2_r[:, :, n, :])
                        # two k-halves for finer dep granularity
                        nc.sync.dma_start(out=xt6_n[n - 1][:, 0:KH, :],
                                          in_=xt6_r[:, 0:KH, n, :])
                        nc.sync.dma_start(out=xt6_n[n - 1][:, KH:KB, :],
                                          in_=xt6_r[:, KH:KB, n, :])
                if xdma is not None:
                    xdma()

                def bf_slice(n, kk):      # kk in 0..KB-1 (k-tile kk+2)
                    if first:
                        if n == 0:
                            return xh_f[kk // KH][:, kk % KH, :]
                        return xt6_n[n - 1][:, kk, :]
                    return src6[:, kk, n * NF:(n + 1) * NF]

                def f8_slice(n):
                    if first:
                        return xt8_n[n]
                    return src8[:, :, n * NF:(n + 1) * NF]

                def b2_slice(n):
                    src = xb2_sb if first else srcb2
                    return src[:, :, n * NF:(n + 1) * NF]

                for n, j in ([(n, j) for n in range(SN) for j in range(DT)]
                             if first else
                             [(n, j) for j in range(DT) for n in range(SN)]):
                    ps = pm.tile([P, NF], F32, tag="mlp")
                    if use8:
                        nc.tensor.matmul(ps, wh_j[j], f8_slice(n),
                                         start=True, stop=False, perf_mode=DR)
                    else:
                        for kk in range(KF8):
                            nc.tensor.matmul(ps, wh_j[j][:, kk, :],
                                             b2_slice(n)[:, kk, :],
                                             start=(kk == 0), stop=False)
                    for kk in range(KB):
                        nc.tensor.matmul(ps, wb_j[j][:, kk, :], bf_slice(n, kk),
                                         start=False, stop=(kk == KB - 1))
                    if isinstance(dst, list):
                        dslice = dst[n][:, j, :]
                    elif len(dst) == 1:
                        dslice = dst[0][:, j, n * NF:(n + 1) * NF]
                    elif j < KF8:
                        dslice = dst[0][:, j, n * NF:(n + 1) * NF]
                    else:
                        dslice = dst[1][:, j - KF8, n * NF:(n + 1) * NF]
                    nc.scalar.activation(out=dslice, in_=ps, func=AF.Tanh,
                                         scale=1.0 / W8SCALE)

            def load_xn():
                xn_r = xn_d.rearrange("(t p) d -> p t d", p=P)
                for t in range(0, ST, 4):
                    nc.sync.dma_start(out=xn_sb[:, t:t + 4, :],
                                      in_=xn_r[:, t:t + 4, :])

            h_for_2 = h8_sb if 2 in FP8_LAYERS else hb2_sb
            h_for_4 = h8_sb if 4 in FP8_LAYERS else hb2_sb
            mlp_layer(1, None, None, None, "wk1", (h_for_2, h6_sb),
                      first=True)
            mlp_layer(2, h6_sb, h8_sb, hb2_sb, "wk2", (kt_sb,))
            mlp_layer(3, None, None, None, "wq1", (h_for_4, h6_sb),
                      xdma=load_xn, first=True)
            mlp_layer(4, h6_sb, h8_sb, hb2_sb, "wq2", qt_n)

        # ---- phase B: scores -> softmax -> transpose -> attended ----
        with tc.tile_pool(name="wexp", bufs=3) as wexp_pool, \
             tc.tile_pool(name="wtT", bufs=3) as wtT_pool, \
             tc.tile_pool(name="sums", bufs=6) as sums_pool, \
             tc.tile_pool(name="outst", bufs=2) as out_pool, \
             tc.tile_pool(name="psum_at", bufs=3, space="PSUM") as pat:

            def scores_softmax_transpose(i):
                wexp = wexp_pool.tile([P, S], BF, tag="wexp")
                sums = sums_pool.tile([P, SN], F32, tag="sums")
                for n in range(SN):
                    ps = psc.tile([P, NF], F32, tag="sc")
                    for k in range(0, DT, 2):
                        nc.tensor.matmul(
                            ps,
                            kt_sb[:, k:k + 2, i * P:(i + 1) * P],
                            qt_n[n][:, k:k + 2, :],
                            start=(k == 0),
                            stop=(k == DT - 2),
                            perf_mode=DR,
                        )
                    # scores are bounded (|sc/32| < ~3): exp without max-shift
                    nc.scalar.activation(
                        out=wexp[:, n * NF:(n + 1) * NF],
                        in_=ps,
                        func=AF.Exp,
                        scale=float(SCALE),
                        accum_out=sums[:, n:n + 1],
                    )
                rcp = sums_pool.tile([P, 1], F32, tag="rcp")
                nc.vector.reduce_sum(rcp, sums, axis=AX)
                nc.vector.reciprocal(rcp, rcp)
                wtT = wtT_pool.tile([P, ST, P], BF, tag="wtT")
                nc.scalar.dma_start_transpose(out=wtT, in_=wexp)
                return wtT, rcp

            def attended(i, wtT, rcp, last=False):
                outst = out_pool.tile([P, D], F32, tag="outst")
                for n in range(DN):
                    ps = pat.tile([P, NF], F32, tag="at")
                    for t in range(ST):
                        nc.tensor.matmul(
                            ps,
                            wtT[:, t, :],
                            xn_sb[:, t, n * NF:(n + 1) * NF],
                            start=(t == 0),
                            stop=(t == ST - 1),
                        )
                    nc.scalar.mul(outst[:, n * NF:(n + 1) * NF], ps, rcp)
                    nc.sync.dma_start(
                        out=out_d[i * P:(i + 1) * P, n * NF:(n + 1) * NF],
                        in_=outst[:, n * NF:(n + 1) * NF],
                    )

            # depth-2 software pipeline: attended(i-2) is emitted after
            # scores(i), giving each transpose a full block of slack
            pipe = []
            for i in range(ST):
                pipe.append(scores_softmax_transpose(i))
                if i >= 2:
                    attended(i - 2, *pipe[i - 2])
            attended(ST - 2, *pipe[ST - 2])
            attended(ST - 1, *pipe[ST - 1], last=True)

    nc.compile()
    return nc


_NC = None


def _get_nc():
    global _NC
    if _NC is None:
        _NC = build_nc()
    return _NC


def _prep_w(w):
    """[d_out, d_in] f32 -> (bf16 [p,j,k2-7,c], fp8 [p,j,k0-1,c],
    bf16 [p,j,k0-1,c]).

    ALL parts are pre-scaled by W8SCALE (so the fp8 tiles sit in e4m3's
    normal range) and the whole psum is uniformly scaled; the tanh
    activation applies 1/W8SCALE. bf16 loses nothing to the scaling."""
    bf16 = ml_dtypes.bfloat16
    f8 = ml_dtypes.float8_e4m3
    wt = np.asarray(w).T.reshape(DT, P, DT, P).transpose(1, 2, 0, 3)  # p j k c
    wt = wt * W8SCALE
    wb = np.ascontiguousarray(wt[:, :, KF8:, :]).astype(bf16)
    w8 = np.ascontiguousarray(wt[:, :, :KF8, :]).astype(f8)
    wb2 = np.ascontiguousarray(wt[:, :, :KF8, :]).astype(bf16)
    return wb, w8, wb2


def make_in_maps(sequence, Kw1, Kw2, Qw1, Qw2):
    bf16 = ml_dtypes.bfloat16
    f8 = ml_dtypes.float8_e4m3
    seq = np.ascontiguousarray(np.transpose(np.asarray(sequence), (1, 0, 2)))
    ws = {}
    for nm, w in (("wk1", Kw1), ("wk2", Kw2), ("wq1", Qw1), ("wq2", Qw2)):
        wb, w8, wb2 = _prep_w(w)
        ws[nm] = wb
        ws[nm + "_8"] = w8
        ws[nm + "_2"] = wb2
    in_maps = []
    for b in range(B):
        xb = seq[b]                                   # [S, D] f32
        xt = np.ascontiguousarray(xb.T)               # [D, S] f32
        # bf16 k-tiles 2-7: [p, kk, s]
        xt6 = np.ascontiguousarray(
            xt[KF8 * P:].reshape(KB, P, S).transpose(1, 0, 2)).astype(bf16)
        # n=0 chunk k-halved: [p, h, kh, f]
        xh = np.ascontiguousarray(
            xt6[:, :, 0:NF].reshape(P, 2, KB // 2, NF))
        # k-tiles 0,1: [p, k, s], fp8 + bf16 fallback
        xklo = xt[:KF8 * P].reshape(KF8, P, S).transpose(1, 0, 2)
        xt8 = np.ascontiguousarray(xklo).astype(f8)
        xb2 = np.ascontiguousarray(xklo).astype(bf16)
        m = {"xn": xb.astype(bf16), "xt6": xt6, "xh": xh, "xt8": xt8,
             "xb2": xb2}
        m.update(ws)
        in_maps.append(m)
    return in_maps


def kernel(sequence, Kw1, Kw2, Qw1, Qw2):
    nc = _get_nc()
    in_maps = make_in_maps(sequence, Kw1, Kw2, Qw1, Qw2)
    res = run_bass_kernel_spmd(nc, in_maps, core_ids=list(range(B)))
    out = np.stack([res.results[b]["out"] for b in range(B)], axis=1)
    return out.astype(np.float32)
